# revision 4
# baseline (speedup 1.0000x reference)
"""Trainium2 Bass kernel for the ergodicity loss.

Math: for x[T=512, B=16, N=32, d=2] in [0,1]^2 and modes (k0,k1) in {0..9}^2:
    basis = cos(pi*k0*x0) * cos(pi*k1*x1)                    (separable!)
    coeffs[b, k0, k1] = sum_{t,n} basis / (T*N) / nf[k1]
    loss = mean((nw * (coeffs - cd))**2)

Device strategy (8 cores, sharded over T: 64 timesteps/core):
  - Per core, per batch: 2048 points = 16 chunks x 128 partitions.
  - C0[p, f, b, k] = cos(pi*k*x0) via: u = (k/2)*x0 (DVE broadcast mult),
    r = round(u + 0.25) (DVE tensor_scalar with int32 output -- fp32->int32
    conversion rounds to nearest on HW), g = u - r (DVE), c = Sin(2*pi*g + pi/2)
    (ACT; arg == 2*pi*(u + 0.25 - r) is in [-pi, pi], inside Sin's valid table
    range; sin(2*pi*(u+0.25)) == cos(pi*k*x)).
  - coeffs partial sums via PE: for each chunk f, block-diagonal matmul
    lhsT = C0 chunk [128, 80] (8 batches x 10 k), rhs = C1 chunk -> PSUM[80, 80]
    accumulated over 16 chunks; 2 groups of 8 batches. Off-diagonal 10x10
    blocks are cross-batch garbage, ignored at gather.
  - Output per core: sout[80, 160] raw PSUM dump (2 groups side by side).
Host: sum the 8 per-core partials, extract diagonal blocks, apply the tiny
[16, 100] normalization + weighted MSE.
"""
import numpy as np

T, B, NA, D = 512, 16, 32, 2
KMAX = 10
NCORES = 8
TLOC = T // NCORES          # 64 timesteps per core
NF = 16                     # point-chunks per batch (64*32/128)
KN = KMAX * KMAX

_STATE = {}


def _np_constants():
    """Replicates reference._constants() exactly in numpy (L = ones)."""
    L = np.ones(D, dtype=np.float32)
    grids = np.meshgrid(*[np.arange(KMAX) for _ in range(D)], indexing="ij")
    K = np.stack(grids, -1).reshape(-1, D).astype(np.float32)          # [100, 2]
    k_scaled = K * np.pi / L
    nf = np.where(K[:, -1] != 0, np.sqrt(L[-1] / 2.0), 1.0).astype(np.float32)
    nw = ((1.0 + (k_scaled ** 2).sum(-1)) ** (-(D + 1) / 2.0) * 100.0).astype(np.float32)
    safe_k = np.where(K != 0, k_scaled, 1.0)
    term = np.where(K != 0,
                    (np.exp(1j * k_scaled * L) - 1.0) / (1j * safe_k * L),
                    1.0 + 0j)
    cd = (term.prod(-1).real / nf).astype(np.float32)                  # [100]
    return nf, nw, cd


def _build(reps: int = 1):
    import concourse.tile as tile
    from concourse import bacc, mybir

    f32 = mybir.dt.float32
    AF = mybir.ActivationFunctionType
    OP = mybir.AluOpType

    nc = bacc.Bacc("TRN2", target_bir_lowering=False, debug=False)
    x0 = nc.dram_tensor("x0", [128, 256], f32, kind="ExternalInput").ap()
    x1 = nc.dram_tensor("x1", [128, 256], f32, kind="ExternalInput").ap()
    kh = nc.dram_tensor("kh", [128, KMAX], f32, kind="ExternalInput").ap()
    sout = nc.dram_tensor("sout", [80, 160], f32, kind="ExternalOutput").ap()

    with tile.TileContext(nc) as tc:
        with tc.tile_pool(name="cpool", bufs=1) as cpool, \
             tc.tile_pool(name="pool", bufs=2) as pool, \
             tc.tile_pool(name="ppool", bufs=2, space="PSUM") as ppool:
            KH = cpool.tile([128, KMAX], f32)
            bias_t = cpool.tile([128, 1], f32)
            scale_t = cpool.tile([128, 1], f32)
            nc.sync.dma_start(KH[:], kh)
            nc.vector.memset(bias_t[:], 0.5 * float(np.pi))
            nc.vector.memset(scale_t[:], 2.0 * float(np.pi))

            for _ in range(reps):
                X0 = pool.tile([128, 256], f32, tag="X0")
                X1 = pool.tile([128, 256], f32, tag="X1")
                C0 = pool.tile([128, 2560], f32, tag="C0")
                C1 = pool.tile([128, 2560], f32, tag="C1")
                R = pool.tile([128, 2560], mybir.dt.int32, tag="R")
                SO = pool.tile([128, 160], f32, tag="SO")

                nc.sync.dma_start(X0[:], x0)
                nc.sync.dma_start(X1[:], x1)

                for X, C in ((X0, C0), (X1, C1)):
                    # u[p, f, b, k] = x[p, f, b] * (k/2)
                    nc.vector.tensor_tensor(
                        C[:].rearrange("p (f b k) -> p f b k", b=16, k=KMAX),
                        X[:].rearrange("p (f b) -> p f b", b=16)
                            .to_broadcast([128, NF, 16, KMAX]),
                        KH[:].unsqueeze(1).unsqueeze(1)
                             .to_broadcast([128, NF, 16, KMAX]),
                        OP.mult)
                    # r = round(u + 0.25)  (int32 write rounds to nearest)
                    nc.vector.tensor_scalar(R[:], C[:], 0.25, None, OP.add)
                    # g = u - r in [-0.5, 0.5] - 0.25
                    nc.vector.tensor_tensor(C[:], C[:], R[:], OP.subtract)
                    # c = sin(2*pi*g + pi/2) = cos(pi*k*x)
                    nc.scalar.activation(C[:], C[:], AF.Sin,
                                         bias=bias_t[:], scale=scale_t[:])

                for g in range(2):
                    ps = ppool.tile([80, 80], f32, tag="ps")
                    for f in range(NF):
                        off = f * 160 + 80 * g
                        nc.tensor.matmul(ps[:], C0[:, off:off + 80],
                                         C1[:, off:off + 80],
                                         start=(f == 0), stop=(f == NF - 1))
                    nc.scalar.copy(SO[0:80, g * 80:(g + 1) * 80], ps[:])

                nc.sync.dma_start(sout, SO[0:80, :])

    nc.compile()
    return nc


def _get_state():
    if "nc" not in _STATE:
        _STATE["nc"] = _build()
    return _STATE["nc"]


def _shard_inputs(x: np.ndarray):
    """x [512, 16, 32, 2] -> per-core {x0, x1 [128, 256], kh [128, 10]}."""
    khalf = np.broadcast_to(
        (np.arange(KMAX, dtype=np.float32) * 0.5), (128, KMAX)).copy()
    in_maps = []
    for c in range(NCORES):
        xc = x[c * TLOC:(c + 1) * TLOC]            # [64, 16, 32, 2]
        arr = xc.reshape(NF, 4, 16, 32, 2)         # (f, tp, b, a, d)
        arr = arr.transpose(1, 3, 0, 2, 4)         # (tp, a, f, b, d)
        arr = arr.reshape(128, 256, 2)             # p = tp*32+a, free = f*16+b
        in_maps.append({
            "x0": np.ascontiguousarray(arr[..., 0]),
            "x1": np.ascontiguousarray(arr[..., 1]),
            "kh": khalf,
        })
    return in_maps


def _gather(souts):
    """souts: list of 8 [80, 160] partials -> scalar loss (float32)."""
    total = np.zeros((80, 160), dtype=np.float64)
    for s in souts:
        total += s.astype(np.float64)
    S = np.empty((B, KMAX, KMAX), dtype=np.float64)
    for g in range(2):
        for bp in range(8):
            r = bp * 10
            S[8 * g + bp] = total[r:r + 10, 80 * g + r:80 * g + r + 10]
    nf, nw, cd = _np_constants()
    coeffs = S.reshape(B, KN) / (NA * T) / nf[None, :].astype(np.float64)
    d = nw[None, :].astype(np.float64) * (coeffs - cd[None, :].astype(np.float64))
    loss = np.mean(d * d)
    return np.float32(loss)


def kernel(x: np.ndarray) -> np.ndarray:
    from concourse.bass_utils import run_bass_kernel_spmd

    nc = _get_state()
    in_maps = _shard_inputs(np.asarray(x, dtype=np.float32))
    res = run_bass_kernel_spmd(nc, in_maps, list(range(NCORES)))
    souts = [r["sout"] for r in res.results]
    return _gather(souts)


# revision 27
# speedup vs baseline: 120.4567x; 120.4567x over previous
"""Trainium2 Bass kernel for the ergodicity loss.

Math: for x[T=512, B=16, N=32, d=2] in [0,1]^2 and modes (k0,k1) in {0..9}^2:
    basis = cos(pi*k0*x0) * cos(pi*k1*x1)                    (separable!)
    coeffs[b, k0, k1] = sum_{t,n} basis / (T*N) / nf[k1]
    loss = mean((nw * (coeffs - cd))**2)

Device strategy (8 cores, data-parallel over T: 64 timesteps/core):
  - Per core, per batch: 2048 points = 16 chunks x 128 partitions.
  - Both coordinate dims processed as ONE fused elementwise stream, in two
    fg-halves h for pipelining (xx[p, h*256 + dd*128 + fg_l*8 + b']):
      w = xx*(k/2) + 16.75       (DVE fused tensor_scalar per (k, h), 2x mode)
        w in [16.75, 21.5): exponent fixed at 4, so the low 19 mantissa bits
        of w hold frac = (v + 0.5 mod 1) * 2^19 where v = x*k/2 + 0.25 and
        v + 0.5 - round(v + 0.5... ) i.e. frac/2^19 - 0.5 = g = v - round(v)
      m = w_bits & 0x7FFFF       (DVE tensor_scalar bitwise_and on int32 view)
      c = Sin(m * 2pi/2^19 - pi) (ACT affine converts back; arg in [-pi, pi)
                                  == Sin's valid table range;
                                  sin(2pi*v) == cos(pi*k*x)), bf16 out
    Quantization of g to 2^-19 adds ~1e-5 abs error to each cos -- negligible
    against the reference after the 16384-point averaging.
  - coeffs partial sums on PE (bf16 inputs, fp32 PSUM): per chunk fg,
    lhsT = C0[:, fg*80:+80] (cols (k0, b')), rhs = C1 same -> PSUM[80, 80]
    accumulated over the 16 f-chunks; 2 batch-groups. Off-diagonal batch
    blocks are garbage, ignored at gather.
  - Output per core: sout[80, 160] raw PSUM dump (2 groups side by side).
Host: sum the 8 per-core partials, extract diagonal blocks, apply the tiny
[16, 100] normalization + weighted MSE.
"""
import numpy as np

T, B, NA, D = 512, 16, 32, 2
KMAX = 10
NCORES = 8
TLOC = T // NCORES          # 64 timesteps per core
NF = 16                     # point-chunks per batch (64*32/128)
KN = KMAX * KMAX

_STATE = {}

# engine for the bitwise-and stage, per (half, dim) quarter
CFG = {"and_eng": ("vector", "vector", "vector", "vector")}


def _np_constants():
    """Replicates reference._constants() exactly in numpy (L = ones)."""
    L = np.ones(D, dtype=np.float32)
    grids = np.meshgrid(*[np.arange(KMAX) for _ in range(D)], indexing="ij")
    K = np.stack(grids, -1).reshape(-1, D).astype(np.float32)          # [100, 2]
    k_scaled = K * np.pi / L
    nf = np.where(K[:, -1] != 0, np.sqrt(L[-1] / 2.0), 1.0).astype(np.float32)
    nw = ((1.0 + (k_scaled ** 2).sum(-1)) ** (-(D + 1) / 2.0) * 100.0).astype(np.float32)
    safe_k = np.where(K != 0, k_scaled, 1.0)
    term = np.where(K != 0,
                    (np.exp(1j * k_scaled * L) - 1.0) / (1j * safe_k * L),
                    1.0 + 0j)
    cd = (term.prod(-1).real / nf).astype(np.float32)                  # [100]
    return nf, nw, cd


def _build(reps: int = 1, loop: bool = False, cfg: dict | None = None):
    import concourse.tile as tile
    from concourse import bacc, mybir

    cfg = {**CFG, **(cfg or {})}
    f32 = mybir.dt.float32
    i32 = mybir.dt.int32
    bf16 = mybir.dt.bfloat16
    AF = mybir.ActivationFunctionType
    OP = mybir.AluOpType

    nc = bacc.Bacc("TRN2", target_bir_lowering=False, debug=False)
    xx = nc.dram_tensor("xx", [128, 512], f32, kind="ExternalInput").ap()
    sout = nc.dram_tensor("sout", [80, 160], f32, kind="ExternalOutput").ap()

    with tile.TileContext(nc) as tc:
        with tc.tile_pool(name="cpool", bufs=1) as cpool, \
             tc.tile_pool(name="pool", bufs=2) as pool, \
             tc.tile_pool(name="ppool", bufs=2, space="PSUM") as ppool:
            scale_t = cpool.tile([128, 1], f32)
            bias_t = cpool.tile([128, 1], f32)
            nc.vector.memset(scale_t[:], 2.0 * float(np.pi) / (1 << 19))
            nc.vector.memset(bias_t[:], -float(np.pi))

            def body(_i=None):
                XX = pool.tile([128, 512], f32, tag="XX")
                U = pool.tile([128, 5120], f32, tag="U")
                C = pool.tile([128, 5120], bf16, tag="C")
                SO = pool.tile([128, 160], f32, tag="SO")
                Ui = U[:].bitcast(i32)

                # one load per half so downstream starts early
                for h in range(2):
                    nc.sync.dma_start(XX[:, 256 * h:256 * (h + 1)],
                                      xx[:, 256 * h:256 * (h + 1)])

                ps = [ppool.tile([80, 80], f32, name=f"ps{g}", tag=f"ps{g}")
                      for g in range(2)]

                for h in range(2):
                    # w = x*(k/2) + 16.75: low 19 mantissa bits hold the
                    # range-reduced phase (w's exponent is pinned to 4)
                    for k in range(KMAX):
                        nc.vector.tensor_scalar(
                            U[:, h * 2560 + k * 256:h * 2560 + (k + 1) * 256],
                            XX[:, h * 256:(h + 1) * 256],
                            0.5 * k, 16.75, OP.mult, OP.add)
                    # m = bits(w) & 0x7FFFF  ->  (g + 0.5) * 2^19
                    nc.vector.tensor_scalar(
                        Ui[:, h * 2560:(h + 1) * 2560],
                        Ui[:, h * 2560:(h + 1) * 2560],
                        0x7FFFF, None, OP.bitwise_and)
                    for dd in range(2):
                        # c = sin(m*2pi/2^19 - pi) = cos(pi*k*x), bf16 out,
                        # scattered into (fg, k, b') so each matmul operand
                        # is one contiguous 80-column slice
                        uin = Ui.rearrange(
                            "p (hh k dd fg b) -> p hh dd k fg b",
                            hh=2, k=KMAX, dd=2, b=8)[:, h, dd]
                        cout = C[:].rearrange(
                            "p (dd hh fg k b) -> p hh dd k fg b",
                            dd=2, hh=2, fg=16, b=8)[:, h, dd]
                        nc.scalar.activation(cout, uin, AF.Sin,
                                             bias=bias_t[:], scale=scale_t[:])

                    # matmuls for this half (both dims' C ready)
                    for fl in range(8):
                        for g in range(2):
                            fg = 16 * h + 2 * fl + g
                            nc.tensor.matmul(
                                ps[g][:],
                                C[:, fg * 80:fg * 80 + 80],
                                C[:, 2560 + fg * 80:2560 + fg * 80 + 80],
                                start=(h == 0 and fl == 0),
                                stop=(h == 1 and fl == 7))

                for g in range(2):
                    nc.scalar.copy(SO[0:80, g * 80:(g + 1) * 80], ps[g][:])
                nc.sync.dma_start(sout, SO[0:80, :])

            if loop:
                with tc.For_i(0, reps, 1) as i:
                    body(i)
            else:
                for _ in range(reps):
                    body()

    nc.compile()
    return nc


def _get_state():
    if "nc" not in _STATE:
        _STATE["nc"] = _build()
    return _STATE["nc"]


def _shard_inputs(x: np.ndarray):
    """x [512, 16, 32, 2] -> per-core {xx [128, 512]}.

    xx free layout: h*256 + d*128 + (fb - 128*h), i.e. per fg-half h the two
    coordinate planes side by side: [x0_h0 | x1_h0 | x0_h1 | x1_h1].
    """
    in_maps = []
    for c in range(NCORES):
        xc = x[c * TLOC:(c + 1) * TLOC]            # [64, 16, 32, 2]
        arr = xc.reshape(NF, 4, 16, 32, 2)         # (f, tp, b, a, d)
        arr = arr.transpose(4, 1, 3, 0, 2)         # (d, tp, a, f, b)
        arr = arr.reshape(2, 128, 256)             # p = tp*32+a, free = f*16+b
        xxc = np.concatenate([arr[0, :, :128], arr[1, :, :128],
                              arr[0, :, 128:], arr[1, :, 128:]], axis=1)
        in_maps.append({"xx": np.ascontiguousarray(xxc)})
    return in_maps


def _gather(souts):
    """souts: list of 8 [80, 160] partials -> scalar loss (float32).

    sout row m = k0*8 + b', col (80*g + k1*8 + b'') for batch b = 8*g + b'.
    """
    total = np.zeros((80, 160), dtype=np.float64)
    for s in souts:
        total += s.astype(np.float64)
    S = np.empty((B, KMAX, KMAX), dtype=np.float64)
    for g in range(2):
        for bp in range(8):
            S[8 * g + bp] = total[bp::8, 80 * g + bp:80 * (g + 1):8]
    nf, nw, cd = _np_constants()
    coeffs = S.reshape(B, KN) / (NA * T) / nf[None, :].astype(np.float64)
    d = nw[None, :].astype(np.float64) * (coeffs - cd[None, :].astype(np.float64))
    loss = np.mean(d * d)
    return np.float32(loss)


def kernel(x: np.ndarray) -> np.ndarray:
    from concourse.bass_utils import run_bass_kernel_spmd

    nc = _get_state()
    in_maps = _shard_inputs(np.asarray(x, dtype=np.float32))
    res = run_bass_kernel_spmd(nc, in_maps, list(range(NCORES)))
    souts = [r["sout"] for r in res.results]
    return _gather(souts)
